# revision 1
# baseline (speedup 1.0000x reference)
"""L2 ECE loss (15-bin histogram binning) on 8 Trainium2 NeuronCores.

Strategy (data-parallel, matches the sharding hint):
  - Shard the N=2^25 element axis contiguously across 8 cores (2^22 each).
  - Per core, stream [128, F] fp32 tiles of confidences/accuracies.
    For each of the 15 bin boundaries t_j (exact f32 values of
    jnp.linspace(0,1,16)), compute in fused single passes:
      * ACT engine:  A_j   = sum(sign(c - t_j))        (fused accum_out)
      * DVE engine:  Td_j  = sum((c > t_j) * (c - a))  (fused accum_out)
    Cumulative counts T_j = (A_j + N)/2; per-bin counts and per-bin
    sum(c-a) follow by adjacent differences.  ECE = sum_b D_b^2/(cnt_b*N).
  - Per-core partial accumulators ([128, ntiles] slots) are DMA'd out and
    the tiny final reduction (3 KB of data) is done on the host in f64.
"""

import numpy as np

import concourse.bass as bass
import concourse.tile as tile
from concourse import bacc, mybir
from concourse import bass_utils

# -- problem constants (hardcoded per spec) ---------------------------------
N_TOTAL = 33554432  # 2**25
N_CORES = 8
NC_PER = N_TOTAL // N_CORES  # 4194304 per core
P = 128
F = 4096  # free-dim elements per tile
T = NC_PER // (P * F)  # 8 tiles per core
N_BINS = 15

# f32 bit patterns of jnp.linspace(0.0, 1.0, 16) — must match the reference
# bit-exactly (jnp.linspace rounds differently from np.linspace).
_BOUND_BITS = [
    0x00000000, 0x3D888889, 0x3E088889, 0x3E4CCCCE,
    0x3E888889, 0x3EAAAAAB, 0x3ECCCCCE, 0x3EEEEEF0,
    0x3F088889, 0x3F19999A, 0x3F2AAAAB, 0x3F3BBBBC,
    0x3F4CCCCE, 0x3F5DDDDF, 0x3F6EEEF0, 0x3F800000,
]
BOUNDS = np.array(_BOUND_BITS, dtype=np.uint32).view(np.float32)

_CACHE = {}
TRACE = False
LAST_RESULTS = None


def _build(repeat=1):
    f32 = mybir.dt.float32
    nc = bacc.Bacc(
        "TRN2",
        target_bir_lowering=False,
        debug=False,
        enable_asserts=False,
        num_devices=N_CORES,
    )
    # pre-register activation bias constants (-t_j) as const APs
    for j in range(N_BINS):
        val = -float(BOUNDS[j])
        if (f32, val) not in nc.const_aps.aps:
            t = nc.alloc_sbuf_tensor(f"const-bias-{j}", [128, 1], f32)
            nc.gpsimd.memset(t.ap(), val)
            nc.const_aps.aps[(f32, val)] = t.ap()
    nc.all_engine_barrier()

    conf = nc.dram_tensor("conf", [NC_PER], f32, kind="ExternalInput").ap()
    acc = nc.dram_tensor("acc", [NC_PER], f32, kind="ExternalInput").ap()
    # accumulator slot layouts: DVE -> [P, T*16] (j=0..14 used),
    # ACT -> [P, T*16] (j=0..14 used)
    out_dve = nc.dram_tensor("out_dve", [P, T * 16], f32, kind="ExternalOutput").ap()
    out_act = nc.dram_tensor("out_act", [P, T * 16], f32, kind="ExternalOutput").ap()

    conf_t = conf.rearrange("(t p f) -> t p f", p=P, f=F)
    acc_t = acc.rearrange("(t p f) -> t p f", p=P, f=F)

    with tile.TileContext(nc) as tc:
        with (
            tc.tile_pool(name="io", bufs=3) as io_pool,
            tc.tile_pool(name="work", bufs=2) as work_pool,
            tc.tile_pool(name="scr", bufs=1) as scr_pool,
            tc.tile_pool(name="accs", bufs=1) as acc_pool,
        ):
            dve_scr = scr_pool.tile([P, F], f32, tag="dve_scr")
            act_scr = scr_pool.tile([P, F], f32, tag="act_scr")
            acc_dve = acc_pool.tile([P, T * 16], f32, tag="acc_dve")
            acc_act = acc_pool.tile([P, T * 16], f32, tag="acc_act")

            for t in range(T * repeat):
                t = t % T
                c = io_pool.tile([P, F], f32, tag="c")
                nc.sync.dma_start(c[:], conf_t[t])
                a = io_pool.tile([P, F], f32, tag="a")
                nc.sync.dma_start(a[:], acc_t[t])

                # d = c - a  (fp32)
                d = work_pool.tile([P, F], f32, tag="d")
                nc.vector.scalar_tensor_tensor(
                    out=d[:],
                    in0=c[:],
                    scalar=0.0,
                    in1=a[:],
                    op0=mybir.AluOpType.bypass,
                    op1=mybir.AluOpType.subtract,
                )

                # DVE: Td_j = sum((c > t_j) * d), fused accumulate
                for j in range(N_BINS):
                    nc.vector.scalar_tensor_tensor(
                        out=dve_scr[:],
                        in0=c[:],
                        scalar=float(BOUNDS[j]),
                        in1=d[:],
                        op0=mybir.AluOpType.is_gt,
                        op1=mybir.AluOpType.mult,
                        accum_out=acc_dve[:, t * 16 + j : t * 16 + j + 1],
                    )

                # ACT: A_j = sum(sign(c - t_j)), fused accumulate
                for j in range(N_BINS):
                    nc.scalar.activation(
                        out=act_scr[:],
                        in_=c[:],
                        func=mybir.ActivationFunctionType.Sign,
                        bias=-float(BOUNDS[j]),
                        scale=1.0,
                        accum_out=acc_act[:, t * 16 + j : t * 16 + j + 1],
                    )

            nc.sync.dma_start(out_dve[:], acc_dve[:])
            nc.sync.dma_start(out_act[:], acc_act[:])

    nc.compile()
    return nc


def kernel(confidences, accuracies):
    global LAST_RESULTS
    conf = np.ascontiguousarray(np.asarray(confidences, dtype=np.float32))
    accu = np.ascontiguousarray(np.asarray(accuracies, dtype=np.float32))
    assert conf.shape == (N_TOTAL,) and accu.shape == (N_TOTAL,)

    if "nc" not in _CACHE:
        _CACHE["nc"] = _build()
    nc = _CACHE["nc"]

    conf_sh = conf.reshape(N_CORES, NC_PER)
    accu_sh = accu.reshape(N_CORES, NC_PER)
    in_maps = [
        {"conf": conf_sh[i], "acc": accu_sh[i]} for i in range(N_CORES)
    ]
    res = bass_utils.run_bass_kernel_spmd(
        nc, in_maps, core_ids=list(range(N_CORES)), trace=TRACE
    )
    LAST_RESULTS = res

    # host-side finish (tiny): combine per-core partial sums in f64
    Td = np.zeros(N_BINS + 1, dtype=np.float64)  # cumulative sum(d) above t_j
    A = np.zeros(N_BINS + 1, dtype=np.float64)  # cumulative sum(sign)
    for r in res.results:
        od = np.asarray(r["out_dve"], dtype=np.float64).reshape(P, T, 16)
        oa = np.asarray(r["out_act"], dtype=np.float64).reshape(P, T, 16)
        Td[:N_BINS] += od.sum(axis=(0, 1))[:N_BINS]
        A[:N_BINS] += oa.sum(axis=(0, 1))[:N_BINS]

    Tcnt = (A[:N_BINS] + N_TOTAL) / 2.0  # counts of {c > t_j}
    Tcnt = np.concatenate([Tcnt, [0.0]])
    Td[N_BINS] = 0.0

    cnt = Tcnt[:N_BINS] - Tcnt[1:]  # per-bin counts
    D = Td[:N_BINS] - Td[1:]  # per-bin sum(c - a)
    with np.errstate(divide="ignore", invalid="ignore"):
        terms = np.where(cnt > 0.5, D * D / np.maximum(cnt, 1.0) / N_TOTAL, 0.0)
    return np.float32(terms.sum())



# revision 14
# speedup vs baseline: 1.2585x; 1.2585x over previous
"""L2 ECE loss (15-bin histogram binning) on 8 Trainium2 NeuronCores.

Strategy (data-parallel over the N=2^25 element axis, 2^22 per core).

Key HW facts driving the design (measured on TRN2):
  - Any DVE op with accum_out lowers to a *_CACHE_REDUCE opcode: 1x only
    (~4.4us per [128,4096] tile).  Non-accum tensor_scalar runs ~1.23us
    (4x), tensor_tensor ~2.29us (2x); scalar_tensor_tensor is always 1x.
  - ACT activation costs ~3.7us/tile regardless of dtype, accum is free.
  - GPSIMD supports plain tensor_tensor (~9us) but no accumulation.
  - PE (tensor engine) is idle: a ones/one-hot matmul reduces a [128,F]
    tile over partitions into a PSUM row at ~2us/tile, accumulating
    across tiles for free.
  - Only gpsimd (SWDGE) DMA can cast f32->fp16 in flight.

Per [128, 4096] tile (F elements per partition):
  - SWDGE DMA-casts c16, a16 (fp16) straight from the f32 DRAM inputs.
  - DVE: d16 = c16 - a16            (tensor_tensor, 2x)
  - ACT j in ACT_J:  sign_j = Sign(c16 - t_j)  [fp16 tile + FREE accum
    S_j = sum(sign_j) -> cnt_j = (S_j + n)/2]
  - DVE j in DVE_J:  mask_j = (c16 > t_j)     (tensor_scalar, 4x)
  - products pd_j = sign_or_mask_j * d16 (tensor_tensor: DVE 2x, 2 on GP)
  - PE: one-hot matmuls reduce each pd_j / mask_j / d16 over partitions
    into row r of a [32, 4096] f32 PSUM accumulator (accumulated over
    all 8 tiles and all 128 partitions).
At the end the PSUM block is copied to SBUF and DMA'd out; the host
reduces the [32, 4096] rows in f64:
  sign path:  D_j = (row_pd_j + Dtot)/2,  cnt_j = (S_j + n)/2
  mask path:  D_j = row_pd_j,             cnt_j = row_mask_j
ECE = sum_b D_b^2/(cnt_b*N) over per-bin adjacent differences.
"""

import numpy as np

import concourse.bass as bass
import concourse.tile as tile
from concourse import bacc, mybir
from concourse import bass_utils

# -- problem constants (hardcoded per spec) ---------------------------------
N_TOTAL = 33554432  # 2**25
N_CORES = 8
NC_PER = N_TOTAL // N_CORES  # 4194304 per core
P = 128
F = 4096  # free-dim elements per tile
T = NC_PER // (P * F)  # 8 tiles per core
N_BINS = 15
NCHUNK = F // 512  # 8 matmul chunks per tile

# f32 bit patterns of jnp.linspace(0.0, 1.0, 16) — must match the reference
# bit-exactly (jnp.linspace rounds differently from np.linspace).
_BOUND_BITS = [
    0x00000000, 0x3D888889, 0x3E088889, 0x3E4CCCCE,
    0x3E888889, 0x3EAAAAAB, 0x3ECCCCCE, 0x3EEEEEF0,
    0x3F088889, 0x3F19999A, 0x3F2AAAAB, 0x3F3BBBBC,
    0x3F4CCCCE, 0x3F5DDDDF, 0x3F6EEEF0, 0x3F800000,
]
BOUNDS = np.array(_BOUND_BITS, dtype=np.uint32).view(np.float32)

# engine assignment per boundary
ACT_J = list(range(0, 10))   # sign tiles + free count accum on ACT
DVE_J = list(range(10, 15))  # 0/1 mask tiles on DVE (counts via PE)
GP_PROD_J = [0, 1]           # products on GPSIMD
# PSUM row layout (32 rows): rows 0..14 = pd_j, 15..19 = mask_j (j-10+15),
# row 20 = d16 (Dtot)
ROW_PD = {j: j for j in range(N_BINS)}
ROW_MASK = {j: 15 + (j - 10) for j in DVE_J}
ROW_DTOT = 20
N_ROWS = 32

_CACHE = {}
TRACE = False
LAST_RESULTS = None


def _build(repeat=1):
    f32 = mybir.dt.float32
    f16 = mybir.dt.float16
    nc = bacc.Bacc(
        "TRN2",
        target_bir_lowering=False,
        debug=False,
        enable_asserts=False,
        num_devices=N_CORES,
    )
    # pre-register activation bias constants (-t_j) as const APs
    for j in ACT_J:
        val = -float(BOUNDS[j])
        if (f32, val) not in nc.const_aps.aps:
            t = nc.alloc_sbuf_tensor(f"const-bias-{j}", [128, 1], f32)
            nc.gpsimd.memset(t.ap(), val)
            nc.const_aps.aps[(f32, val)] = t.ap()
    nc.all_engine_barrier()

    conf = nc.dram_tensor("conf", [NC_PER], f32, kind="ExternalInput").ap()
    acc = nc.dram_tensor("acc", [NC_PER], f32, kind="ExternalInput").ap()
    out_sgn = nc.dram_tensor("out_sgn", [P, T * 16], f32, kind="ExternalOutput").ap()
    out_rows = nc.dram_tensor("out_rows", [N_ROWS, F], f32, kind="ExternalOutput").ap()

    conf_t = conf.rearrange("(t p f) -> t p f", p=P, f=F)
    acc_t = acc.rearrange("(t p f) -> t p f", p=P, f=F)

    with tile.TileContext(nc) as tc:
        with (
            tc.tile_pool(name="io", bufs=2) as io_pool,
            tc.tile_pool(name="work", bufs=2) as work_pool,
            tc.tile_pool(name="sgn", bufs=2) as sgn_pool,
            tc.tile_pool(name="prod", bufs=3) as prod_pool,
            tc.tile_pool(name="scr", bufs=1) as scr_pool,
            tc.tile_pool(name="accs", bufs=1) as acc_pool,
            tc.psum_pool(name="ps", bufs=1) as psum_pool,
        ):
            # one-hot stationary matrices: ohall[:, r*32 + m] = (m == r)
            ohall = scr_pool.tile([P, N_ROWS * N_ROWS], f16, tag="ohall")
            nc.vector.memset(ohall[:], 0.0)
            for r in range(N_ROWS):
                nc.gpsimd.memset(ohall[:, r * N_ROWS + r : r * N_ROWS + r + 1], 1.0)

            psum = psum_pool.tile([N_ROWS, F], f32, tag="psum")
            acc_sgn = acc_pool.tile([P, T * 16], f32, tag="acc_sgn")
            rows_sb = acc_pool.tile([N_ROWS, F], f32, tag="rows_sb")

            # per-chunk accumulation bookkeeping for start= flags
            started = [False] * NCHUNK
            n_reduce_total = (T * repeat) * (N_BINS + len(DVE_J) + 1)
            emitted = [0]

            def pe_reduce(src_tile, row):
                """Accumulate per-column partition-sums of src into psum row."""
                emitted[0] += 1
                last = emitted[0] == n_reduce_total
                for k in range(NCHUNK):
                    nc.tensor.matmul(
                        psum[:, k * 512 : (k + 1) * 512],
                        ohall[:, row * N_ROWS : (row + 1) * N_ROWS],
                        src_tile[:, k * 512 : (k + 1) * 512],
                        start=not started[k],
                        stop=last,
                        skip_group_check=True,
                    )
                    started[k] = True

            for t in range(T * repeat):
                t = t % T
                c16 = io_pool.tile([P, F], f16, tag="c16")
                nc.gpsimd.dma_start(c16[:], conf_t[t])
                a16 = io_pool.tile([P, F], f16, tag="a16")
                nc.gpsimd.dma_start(a16[:], acc_t[t])

                d16 = work_pool.tile([P, F], f16, tag="d16")
                nc.vector.tensor_tensor(
                    out=d16[:], in0=c16[:], in1=a16[:], op=mybir.AluOpType.subtract
                )
                pe_reduce(d16, ROW_DTOT)

                # ACT: sign tiles + free count accum
                sgn_tiles = {}
                for j in ACT_J:
                    s = sgn_pool.tile([P, F], f16, tag=f"sgn{j % 2}")
                    nc.scalar.activation(
                        out=s[:],
                        in_=c16[:],
                        func=mybir.ActivationFunctionType.Sign,
                        bias=-float(BOUNDS[j]),
                        scale=1.0,
                        accum_out=acc_sgn[:, t * 16 + j : t * 16 + j + 1],
                    )
                    sgn_tiles[j] = s
                    # product immediately after each sign tile
                    pd = prod_pool.tile([P, F], f16, tag="pd")
                    if j in GP_PROD_J:
                        nc.gpsimd.tensor_tensor(
                            out=pd[:], in0=s[:], in1=d16[:], op=mybir.AluOpType.mult
                        )
                    else:
                        nc.vector.tensor_tensor(
                            out=pd[:], in0=s[:], in1=d16[:], op=mybir.AluOpType.mult
                        )
                    pe_reduce(pd, ROW_PD[j])

                # DVE: 0/1 mask tiles (ts 4x) + products
                for j in DVE_J:
                    m = sgn_pool.tile([P, F], f16, tag="msk")
                    nc.vector.tensor_scalar(
                        out=m[:],
                        in0=c16[:],
                        scalar1=float(BOUNDS[j]),
                        scalar2=1.0,
                        op0=mybir.AluOpType.is_gt,
                        op1=mybir.AluOpType.mult,
                    )
                    pe_reduce(m, ROW_MASK[j])
                    pd = prod_pool.tile([P, F], f16, tag="pd")
                    nc.vector.tensor_tensor(
                        out=pd[:], in0=m[:], in1=d16[:], op=mybir.AluOpType.mult
                    )
                    pe_reduce(pd, ROW_PD[j])

            # extract PSUM block -> SBUF -> DRAM (host does exact f64 sums)
            nc.vector.tensor_scalar(
                out=rows_sb[:],
                in0=psum[:],
                scalar1=1.0,
                scalar2=None,
                op0=mybir.AluOpType.mult,
            )
            nc.sync.dma_start(out_rows[:], rows_sb[:])
            nc.sync.dma_start(out_sgn[:], acc_sgn[:])

    nc.compile()
    return nc


def kernel(confidences, accuracies):
    global LAST_RESULTS
    conf = np.ascontiguousarray(np.asarray(confidences, dtype=np.float32))
    accu = np.ascontiguousarray(np.asarray(accuracies, dtype=np.float32))
    assert conf.shape == (N_TOTAL,) and accu.shape == (N_TOTAL,)

    if "nc" not in _CACHE:
        _CACHE["nc"] = _build()
    nc = _CACHE["nc"]

    conf_sh = conf.reshape(N_CORES, NC_PER)
    accu_sh = accu.reshape(N_CORES, NC_PER)
    in_maps = [
        {"conf": conf_sh[i], "acc": accu_sh[i]} for i in range(N_CORES)
    ]
    res = bass_utils.run_bass_kernel_spmd(
        nc, in_maps, core_ids=list(range(N_CORES)), trace=TRACE
    )
    LAST_RESULTS = res

    # host-side finish (tiny): combine per-core partials in f64
    CA = np.zeros(N_BINS + 1, dtype=np.float64)  # counts of {c16 > t_j}
    D = np.zeros(N_BINS + 1, dtype=np.float64)  # cumulative sum(d16) above t_j
    for r in res.results:
        sgn = np.asarray(r["out_sgn"], dtype=np.float64).reshape(P, T, 16)
        rows = np.asarray(r["out_rows"], dtype=np.float64).reshape(N_ROWS, F)
        sgn_sums = sgn.sum(axis=(0, 1))
        row_sums = rows.sum(axis=1)
        dtot = row_sums[ROW_DTOT]
        for j in ACT_J:
            CA[j] += (sgn_sums[j] + NC_PER) / 2.0
            D[j] += (row_sums[ROW_PD[j]] + dtot) / 2.0
        for j in DVE_J:
            CA[j] += row_sums[ROW_MASK[j]]
            D[j] += row_sums[ROW_PD[j]]

    CA[N_BINS] = 0.0
    D[N_BINS] = 0.0

    cnt = CA[:N_BINS] - CA[1:]  # per-bin counts
    Db = D[:N_BINS] - D[1:]  # per-bin sum(c - a)
    with np.errstate(divide="ignore", invalid="ignore"):
        terms = np.where(cnt > 0.5, Db * Db / np.maximum(cnt, 1.0) / N_TOTAL, 0.0)
    return np.float32(terms.sum())


# revision 19
# speedup vs baseline: 1.6039x; 1.2745x over previous
"""L2 ECE loss (15-bin histogram binning) on 8 Trainium2 NeuronCores.

Strategy (data-parallel over the N=2^25 element axis, 2^22 per core).

Key HW facts driving the design (measured on TRN2):
  - Any DVE op with accum_out lowers to a *_CACHE_REDUCE opcode: 1x only
    (~4.4us per [128,4096] tile).  Non-accum tensor_scalar runs ~1.23us
    (4x), tensor_tensor ~2.29us (2x); scalar_tensor_tensor is always 1x.
  - ACT activation costs ~3.7us/tile regardless of dtype, accum is free.
  - GPSIMD supports plain tensor_tensor (~9us) but no accumulation.
  - PE (tensor engine) is idle: a ones/one-hot matmul reduces a [128,F]
    tile over partitions into a PSUM row at ~2us/tile, accumulating
    across tiles for free.
  - Only gpsimd (SWDGE) DMA can cast f32->fp16 in flight.

Per [128, 4096] tile (F elements per partition):
  - SWDGE DMA-casts c16, a16 (fp16) straight from the f32 DRAM inputs.
  - DVE: d16 = c16 - a16            (tensor_tensor, 2x)
  - ACT j in ACT_J:  sign_j = Sign(c16 - t_j)  [fp16 tile + FREE accum
    S_j = sum(sign_j) -> cnt_j = (S_j + n)/2]
  - DVE j in DVE_J:  mask_j = (c16 > t_j)     (tensor_scalar, 4x)
  - products pd_j = sign_or_mask_j * d16 (tensor_tensor: DVE 2x, 2 on GP)
  - PE: one-hot matmuls reduce each pd_j / mask_j / d16 over partitions
    into row r of a [32, 4096] f32 PSUM accumulator (accumulated over
    all 8 tiles and all 128 partitions).
At the end the PSUM block is copied to SBUF and DMA'd out; the host
reduces the [32, 4096] rows in f64:
  sign path:  D_j = (row_pd_j + Dtot)/2,  cnt_j = (S_j + n)/2
  mask path:  D_j = row_pd_j,             cnt_j = row_mask_j
ECE = sum_b D_b^2/(cnt_b*N) over per-bin adjacent differences.
"""

import numpy as np

import concourse.bass as bass
import concourse.tile as tile
from concourse import bacc, mybir
from concourse import bass_utils

# -- problem constants (hardcoded per spec) ---------------------------------
N_TOTAL = 33554432  # 2**25
N_CORES = 8
NC_PER = N_TOTAL // N_CORES  # 4194304 per core
P = 128
F = 4096  # free-dim elements per tile
T = NC_PER // (P * F)  # 8 tiles per core
N_BINS = 15
NCHUNK = F // 512  # 8 matmul chunks per tile

# f32 bit patterns of jnp.linspace(0.0, 1.0, 16) — must match the reference
# bit-exactly (jnp.linspace rounds differently from np.linspace).
_BOUND_BITS = [
    0x00000000, 0x3D888889, 0x3E088889, 0x3E4CCCCE,
    0x3E888889, 0x3EAAAAAB, 0x3ECCCCCE, 0x3EEEEEF0,
    0x3F088889, 0x3F19999A, 0x3F2AAAAB, 0x3F3BBBBC,
    0x3F4CCCCE, 0x3F5DDDDF, 0x3F6EEEF0, 0x3F800000,
]
BOUNDS = np.array(_BOUND_BITS, dtype=np.uint32).view(np.float32)

# engine assignment per boundary
ACT_J = list(range(0, 11))   # sign tiles + free count accum on ACT
DVE_J = list(range(11, 15))  # 0/1 mask tiles on DVE (counts via PE)
# j = 0: sign_0 is all-ones (c16 > 0), so pd_0 == d16; the d16 row serves
# as both Dtot and row_pd_0 — no product op for j=0.
# PSUM row layout (32 rows): row 0 = d16 (= Dtot = pd_0), rows 1..14 = pd_j,
# rows 15..18 = mask_j for j in 11..14
ROW_PD = {j: j for j in range(N_BINS)}  # row 0 is d16/pd_0
ROW_MASK = {j: 15 + (j - 11) for j in DVE_J}
ROW_DTOT = 0
N_ROWS = 32
PSF = 512  # psum accumulator free size (one bank); all chunks fold into it

_CACHE = {}
TRACE = False
LAST_RESULTS = None


def _build(repeat=1):
    f32 = mybir.dt.float32
    f16 = mybir.dt.float16
    nc = bacc.Bacc(
        "TRN2",
        target_bir_lowering=False,
        debug=False,
        enable_asserts=False,
        num_devices=N_CORES,
    )
    # pre-register activation bias constants (-t_j) as const APs
    for j in ACT_J:
        val = -float(BOUNDS[j])
        if (f32, val) not in nc.const_aps.aps:
            t = nc.alloc_sbuf_tensor(f"const-bias-{j}", [128, 1], f32)
            nc.gpsimd.memset(t.ap(), val)
            nc.const_aps.aps[(f32, val)] = t.ap()
    nc.all_engine_barrier()

    conf = nc.dram_tensor("conf", [NC_PER], f32, kind="ExternalInput").ap()
    acc = nc.dram_tensor("acc", [NC_PER], f32, kind="ExternalInput").ap()
    out_sgn = nc.dram_tensor("out_sgn", [P, T * 16], f32, kind="ExternalOutput").ap()
    out_rows = nc.dram_tensor("out_rows", [N_ROWS, PSF], f32, kind="ExternalOutput").ap()

    conf_t = conf.rearrange("(t p f) -> t p f", p=P, f=F)
    acc_t = acc.rearrange("(t p f) -> t p f", p=P, f=F)

    with tile.TileContext(nc) as tc:
        with (
            tc.tile_pool(name="io", bufs=2) as io_pool,
            tc.tile_pool(name="work", bufs=2) as work_pool,
            tc.tile_pool(name="sgn", bufs=2) as sgn_pool,
            tc.tile_pool(name="prod", bufs=3) as prod_pool,
            tc.tile_pool(name="scr", bufs=1) as scr_pool,
            tc.tile_pool(name="accs", bufs=1) as acc_pool,
            tc.psum_pool(name="ps", bufs=1) as psum_pool,
        ):
            # one-hot stationary matrices: ohall[:, r*32 + m] = (m == r)
            ohall = scr_pool.tile([P, N_ROWS * N_ROWS], f16, tag="ohall")
            nc.vector.memset(ohall[:], 0.0)
            for r in range(N_ROWS):
                nc.gpsimd.memset(ohall[:, r * N_ROWS + r : r * N_ROWS + r + 1], 1.0)

            psum = psum_pool.tile([N_ROWS, PSF], f32, tag="psum")
            acc_sgn = acc_pool.tile([P, T * 16], f32, tag="acc_sgn")
            rows_sb = acc_pool.tile([N_ROWS, PSF], f32, tag="rows_sb")

            # accumulation bookkeeping for start/stop flags
            started = [False]
            n_reduce_total = (T * repeat) * (N_BINS - 1 + len(DVE_J) + 1)
            emitted = [0]

            def pe_reduce(src_tile, row):
                """Accumulate partition+chunk sums of src into psum row."""
                emitted[0] += 1
                last = emitted[0] == n_reduce_total
                for k in range(NCHUNK):
                    nc.tensor.matmul(
                        psum[:],
                        ohall[:, row * N_ROWS : (row + 1) * N_ROWS],
                        src_tile[:, k * 512 : (k + 1) * 512],
                        start=not started[0],
                        stop=last and k == NCHUNK - 1,
                        skip_group_check=True,
                    )
                    started[0] = True

            for t in range(T * repeat):
                t = t % T
                c16 = io_pool.tile([P, F], f16, tag="c16")
                nc.gpsimd.dma_start(c16[:], conf_t[t])
                a16 = io_pool.tile([P, F], f16, tag="a16")
                nc.gpsimd.dma_start(a16[:], acc_t[t])

                d16 = work_pool.tile([P, F], f16, tag="d16")
                nc.vector.tensor_tensor(
                    out=d16[:], in0=c16[:], in1=a16[:], op=mybir.AluOpType.subtract
                )
                pe_reduce(d16, ROW_DTOT)

                # ACT: sign tiles + free count accum
                for j in ACT_J:
                    s = sgn_pool.tile([P, F], f16, tag=f"sgn{j % 2}")
                    nc.scalar.activation(
                        out=s[:],
                        in_=c16[:],
                        func=mybir.ActivationFunctionType.Sign,
                        bias=-float(BOUNDS[j]),
                        scale=1.0,
                        accum_out=acc_sgn[:, t * 16 + j : t * 16 + j + 1],
                    )
                    if j == 0:
                        continue  # pd_0 == d16 (sign_0 is all-ones)
                    pd = prod_pool.tile([P, F], f16, tag="pd")
                    nc.vector.tensor_tensor(
                        out=pd[:], in0=s[:], in1=d16[:], op=mybir.AluOpType.mult
                    )
                    pe_reduce(pd, ROW_PD[j])

                # DVE: 0/1 mask tiles (ts 4x) + products
                for j in DVE_J:
                    m = sgn_pool.tile([P, F], f16, tag="msk")
                    nc.vector.tensor_scalar(
                        out=m[:],
                        in0=c16[:],
                        scalar1=float(BOUNDS[j]),
                        scalar2=1.0,
                        op0=mybir.AluOpType.is_gt,
                        op1=mybir.AluOpType.mult,
                    )
                    pe_reduce(m, ROW_MASK[j])
                    pd = prod_pool.tile([P, F], f16, tag="pd")
                    nc.vector.tensor_tensor(
                        out=pd[:], in0=m[:], in1=d16[:], op=mybir.AluOpType.mult
                    )
                    pe_reduce(pd, ROW_PD[j])

            # extract PSUM block -> SBUF -> DRAM (host does exact f64 sums)
            nc.vector.tensor_scalar(
                out=rows_sb[:],
                in0=psum[:],
                scalar1=1.0,
                scalar2=None,
                op0=mybir.AluOpType.mult,
            )
            nc.sync.dma_start(out_rows[:], rows_sb[:])
            nc.sync.dma_start(out_sgn[:], acc_sgn[:])

    nc.compile()
    return nc


def kernel(confidences, accuracies):
    global LAST_RESULTS
    conf = np.ascontiguousarray(np.asarray(confidences, dtype=np.float32))
    accu = np.ascontiguousarray(np.asarray(accuracies, dtype=np.float32))
    assert conf.shape == (N_TOTAL,) and accu.shape == (N_TOTAL,)

    if "nc" not in _CACHE:
        _CACHE["nc"] = _build()
    nc = _CACHE["nc"]

    conf_sh = conf.reshape(N_CORES, NC_PER)
    accu_sh = accu.reshape(N_CORES, NC_PER)
    in_maps = [
        {"conf": conf_sh[i], "acc": accu_sh[i]} for i in range(N_CORES)
    ]
    res = bass_utils.run_bass_kernel_spmd(
        nc, in_maps, core_ids=list(range(N_CORES)), trace=TRACE
    )
    LAST_RESULTS = res

    # host-side finish (tiny): combine per-core partials in f64
    CA = np.zeros(N_BINS + 1, dtype=np.float64)  # counts of {c16 > t_j}
    D = np.zeros(N_BINS + 1, dtype=np.float64)  # cumulative sum(d16) above t_j
    for r in res.results:
        sgn = np.asarray(r["out_sgn"], dtype=np.float64).reshape(P, T, 16)
        rows = np.asarray(r["out_rows"], dtype=np.float64).reshape(N_ROWS, PSF)
        sgn_sums = sgn.sum(axis=(0, 1))
        row_sums = rows.sum(axis=1)
        dtot = row_sums[ROW_DTOT]
        for j in ACT_J:
            CA[j] += (sgn_sums[j] + NC_PER) / 2.0
            D[j] += (row_sums[ROW_PD[j]] + dtot) / 2.0
        for j in DVE_J:
            CA[j] += row_sums[ROW_MASK[j]]
            D[j] += row_sums[ROW_PD[j]]

    CA[N_BINS] = 0.0
    D[N_BINS] = 0.0

    cnt = CA[:N_BINS] - CA[1:]  # per-bin counts
    Db = D[:N_BINS] - D[1:]  # per-bin sum(c - a)
    with np.errstate(divide="ignore", invalid="ignore"):
        terms = np.where(cnt > 0.5, Db * Db / np.maximum(cnt, 1.0) / N_TOTAL, 0.0)
    return np.float32(terms.sum())
